# revision 36
# baseline (speedup 1.0000x reference)
"""Trainium2 Bass kernel: parity-polynomial segment_reduce (one-hot form).

Reference math:
    spins = 1 - 2*bits                                   # {-1,+1}
    parities[b,t] = prod_o spins_pad[b, idx_pad[t,o]]    # [B, T]
    out[b] = parities[b] @ theta

Every parity factor is (-1)^{bit}, so with mask[t] = XOR-fold of idx_pad[t]
bit positions (pad index contributes nothing, repeats cancel):

    out[b] = f(key_b),   f = WHT_4096(theta scattered by mask)

where key_b is the 12-bit key from bits 0..11 (all masks span bits 0..11).
Split key = kh*64 + kl.  Then out[b] = F[kl_b, kh_b] with
F[k,j] = sum_{ml,mh} Theta[mh,ml] * H64[ml,k] * H64[mh,j].

On device (per core, batch-sharded 512 rows, 4 chunks of 128 samples):

  1. key offsets by matmul: M1[k,b] = kl(b)-k  (augmented ones row carries
     the -k), and transposed M2T[b,j] = kh(b)-j per 128-sample chunk
     (stationary = bits chunk), sharing one [7,64] pattern matrix.
  2. one-hots via DVE is_equal against immediate 0.0 (proven ISA form).
  3. F chain (theta-gated, parallel to 1-2): B = Theta^T-contracted H,
     F = H-contracted B -- two small matmuls with ACT-engine PSUM->SBUF
     bf16 copies between, all off the bits critical path.
  4. RT[b,j] = sum_k F[k,j]*ohlo[k,b]: 4 matmuls with the one-hot chunk
     as the stationary operand, so RT lands batch-major [128, 4*64].
  5. out = sum_j RT * ohhiT: DVE tensor_mul + free-dim tensor_reduce
     -> y[128,4] partition-major, sample g*128+j on partition j, col g.
  6. store via SWDGE kv_writeback whose descriptors are PREPARED during
     the input-DMA wait; the trigger (manually gated on a semaphore the
     reduce's follower bumps) skips the HWDGE and DGE-delay latencies of
     a fresh DMA.  An SP wait on the DMA-completion semaphore gates
     kernel exit.

PE warm-up matmuls (junk outer products) run during the input DMA wait so
the real matmuls hit a warm PE pipeline on hardware.

Host does only sharding, dtype/layout staging, and index bookkeeping
(mask XOR-fold + theta scatter).  All theta- and bit-dependent arithmetic
runs on device.
"""

import numpy as np

B, NUM_BITS, ORDER = 4096, 32, 12
N_CORES = 8
B_LOCAL = B // N_CORES          # 512
KEYS = 1 << ORDER               # 4096
M = 64                          # 2^6: half-key alphabet
HALF = 6
ROWS = HALF + 1                 # 6 bit rows + ones row
CHUNK = 128                     # samples per transposed chunk
N_CHUNK = B_LOCAL // CHUNK      # 4
TB_COLS = 2 * B_LOCAL + M       # bits-hi | bits-lo | pattern matrix
N_WARM = 18                     # PE warm-up matmuls (128 cols each)

_STATE = {}


def _sylvester(n):
    """H[i,j] = (-1)^popcount(i&j), Sylvester ordering."""
    h = np.array([[1.0]], dtype=np.float32)
    while h.shape[0] < n:
        h = np.block([[h, h], [h, -h]])
    return np.ascontiguousarray(h, dtype=np.float32)


def _build_module():
    import concourse.mybir as mybir
    import concourse.tile as tile
    from concourse import bacc

    f32 = mybir.dt.float32
    bf16 = mybir.dt.bfloat16
    i32 = mybir.dt.int32
    nc = bacc.Bacc(
        "TRN2",
        target_bir_lowering=False,
        debug=False,
        enable_asserts=True,
        num_devices=N_CORES,
    )

    # tb [7, 1088]: rows 0..5 bits (cols 0..511 hi = bits 6..11,
    # cols 512..1023 lo = bits 0..5), row 6 = ones; cols 1024..1087 the
    # shared pattern matrix P[i,m] = 1<<i, P[6,m] = -m.
    tb = nc.dram_tensor("tb", [ROWS, TB_COLS], bf16, kind="ExternalInput").ap()
    # tt [64, 128]: cols 0..63 Theta[mh,ml] = theta_spread[mh*64+ml],
    # cols 64..127 H64 (Sylvester, +-1).
    tt = nc.dram_tensor("tt", [M, 2 * M], bf16, kind="ExternalInput").ap()
    # kv_writeback layout [batch=1, d_head_inner=128, d_head_outer=1, n_ctx=4]
    out = nc.dram_tensor("out", [1, CHUNK, 1, N_CHUNK], f32, kind="ExternalOutput").ap()

    with tile.TileContext(nc) as tc:
        with (
            tc.tile_pool(name="sb", bufs=1) as sb,
            tc.tile_pool(name="ps", bufs=1, space="PSUM") as ps,
        ):
            # --- warm-up fuel + output index metadata (no input deps) ---
            t_wu = sb.tile([1, CHUNK], bf16)
            nc.vector.memset(t_wu, 1.0)
            t_ctx = sb.tile([CHUNK, 1], i32)
            nc.gpsimd.memset(t_ctx, 0)

            # --- input DMAs: bits first, theta second ---
            t_b = sb.tile([ROWS, TB_COLS], bf16)
            nc.sync.dma_start(out=t_b, in_=tb)
            t_t = sb.tile([M, 2 * M], bf16)
            nc.sync.dma_start(out=t_t, in_=tt)

            # --- result tile + prepared output descriptors ---
            # Emitted before t_y has any writer, so the prep's only sync dep
            # is the t_ctx memset and it runs during the input-DMA wait.
            t_y = sb.tile([CHUNK, N_CHUNK], f32)
            dma_sem = nc.alloc_semaphore("swdge_dma")
            sem_y = nc.alloc_semaphore("y_ready")
            y2d = t_y[:, :]
            from concourse.ap import AP as _AP

            # [dhi=128, dho=1, batch=1, ncn=4]; dho stride 4 so batch_step=1
            in4d = _AP(
                y2d.tensor,
                y2d.offset,
                [list(y2d.ap[0]), [N_CHUNK, 1], [N_CHUNK, 1], [1, N_CHUNK]],
            )
            nc.gpsimd.kv_writeback(
                out, in4d, t_ctx[:, :], prepare_only=True, sem=dma_sem
            )

            # --- PSUM pool: 8 banks / 16KB per partition; share banks ---
            p_M1 = ps.tile([M, B_LOCAL], f32)
            p_M2T = ps.tile([CHUNK, N_CHUNK * M], f32)
            p_BF = ps.tile([M, 2 * M], f32)
            p_RT = ps.tile([CHUNK, N_CHUNK * M], f32)

            # --- PE warm-up: junk outer products while the DMA flies
            # (into p_M2T, overwritten by the real matmuls later) ---
            for _ in range(N_WARM):
                nc.tensor.matmul(p_M2T[:, 0:CHUNK], t_wu, t_wu)

            bits_hi = t_b[:, 0:B_LOCAL]
            bits_lo = t_b[:, B_LOCAL : 2 * B_LOCAL]
            pat = t_b[:, 2 * B_LOCAL : 2 * B_LOCAL + M]
            t_Th = t_t[:, 0:M]
            t_H = t_t[:, M : 2 * M]

            # --- key-offset matmuls ---
            nc.tensor.matmul(p_M1, pat, bits_lo)           # kl(b) - k
            for g in range(N_CHUNK):
                nc.tensor.matmul(
                    p_M2T[:, g * M : (g + 1) * M],
                    bits_hi[:, g * CHUNK : (g + 1) * CHUNK],
                    pat,
                )                                          # kh(b) - j

            # --- F chain (theta-gated; ACT copies keep DVE free; GPSIMD
            # cannot access PSUM) ---
            nc.tensor.matmul(p_BF[:, 0:M], t_Th, t_H)      # B[ml,j]
            t_B = sb.tile([M, M], bf16)
            nc.scalar.copy(t_B, p_BF[:, 0:M])
            nc.tensor.matmul(p_BF[:, M : 2 * M], t_H, t_B)  # F[k,j]
            t_F = sb.tile([M, M], bf16)
            nc.scalar.copy(t_F, p_BF[:, M : 2 * M])

            # --- one-hots (DVE is_equal vs immediate 0.0) ---
            def onehot(dst, src):
                nc.vector.tensor_scalar(
                    out=dst,
                    in0=src,
                    scalar1=0.0,
                    scalar2=None,
                    op0=mybir.AluOpType.is_equal,
                )

            t_ohlo = sb.tile([M, B_LOCAL], bf16)
            onehot(t_ohlo, p_M1)
            t_ohhiT = sb.tile([CHUNK, N_CHUNK * M], bf16)
            onehot(t_ohhiT, p_M2T)

            # --- RT[b,j] = F[kl(b), j] via stationary one-hot chunks ---
            for g in range(N_CHUNK):
                nc.tensor.matmul(
                    p_RT[:, g * M : (g + 1) * M],
                    t_ohlo[:, g * CHUNK : (g + 1) * CHUNK],
                    t_F,
                )

            # --- combine + reduce -> y[j, g] = out[g*128 + j] ---
            t_prod = sb.tile([CHUNK, N_CHUNK * M], bf16)
            nc.vector.tensor_mul(t_prod, p_RT, t_ohhiT)
            red = nc.vector.tensor_reduce(
                out=t_y[:, :],
                in_=t_prod[:, :].rearrange("p (g m) -> p g m", m=M),
                axis=mybir.AxisListType.X,
                op=mybir.AluOpType.add,
            )
            # separate DVE sem bump (engine ops can't carry 2 updates); the
            # sync edge pins it after the reduce so the scheduler can't hoist
            import bass_rust as _br

            si = nc.vector.sem_inc(sem_y, 1)
            _deps = _br.InstructionNameOrderedSet()
            _deps.add(red.ins.name)
            si.ins.add_sync_dependencies_from(_deps)

            # --- fire the prepared writeback once t_y is real ---
            nc.gpsimd.trigger_dma(count=1)._wait_ge(sem_y, 1)
            # gate kernel exit on the writeback landing in DRAM (idle SP)
            nc.sync.wait_ge(dma_sem, 16)

    nc.compile()
    return nc


def _get_module():
    nc = _STATE.get("nc")
    if nc is None:
        nc = _build_module()
        _STATE["nc"] = nc
    return nc


def _host_prep(bitstrings, theta, idx_pad):
    """Index bookkeeping + input staging. Returns per-core input maps."""
    import ml_dtypes

    bitstrings = np.asarray(bitstrings)
    theta = np.asarray(theta, dtype=np.float32)
    idx_pad = np.asarray(idx_pad).astype(np.int64)

    # mask[t] = XOR-fold of one-hot bit positions (pad index >= NUM_BITS -> no bit)
    onehots = np.where(idx_pad >= NUM_BITS, 0, np.int64(1) << np.clip(idx_pad, 0, 62))
    masks = np.bitwise_xor.reduce(onehots, axis=1)
    if masks.size and int(masks.max()) >= KEYS:
        raise NotImplementedError(
            "kernel specialized for masks spanning bits 0..11 "
            f"(max mask {int(masks.max())})"
        )
    theta_spread = np.zeros(KEYS, np.float32)
    np.add.at(theta_spread, masks, theta)

    # Theta[mh, ml] | H64
    ttbuf = np.zeros((M, 2 * M), np.float32)
    ttbuf[:, 0:M] = theta_spread.reshape(M, M)
    ttbuf[:, M : 2 * M] = _sylvester(M)
    tt = ttbuf.astype(ml_dtypes.bfloat16)

    # shared pattern matrix: P[i, m] = 1<<i, P[6, m] = -m
    pat = np.zeros((ROWS, M), np.float32)
    for i in range(HALF):
        pat[i, :] = float(1 << i)
    pat[HALF, :] = -np.arange(M, dtype=np.float32)

    bits_f = bitstrings.astype(np.float32)
    in_maps = []
    for c in range(N_CORES):
        bl = bits_f[c * B_LOCAL : (c + 1) * B_LOCAL, :]  # [512, 32]
        tbuf = np.ones((ROWS, TB_COLS), np.float32)
        tbuf[0:HALF, 0:B_LOCAL] = bl[:, HALF:ORDER].T             # bits 6..11
        tbuf[0:HALF, B_LOCAL : 2 * B_LOCAL] = bl[:, 0:HALF].T     # bits 0..5
        tbuf[:, 2 * B_LOCAL :] = pat
        in_maps.append({"tb": tbuf.astype(ml_dtypes.bfloat16), "tt": tt})
    return in_maps


def _unpack_out(arr):
    """[1,128,1,4] device layout -> [512] local outputs (b = g*128 + j)."""
    a = np.asarray(arr, dtype=np.float32).reshape(CHUNK, N_CHUNK)
    return a.T.reshape(-1)


def kernel(bitstrings, theta, idx_pad):
    from concourse.bass_utils import run_bass_kernel_spmd

    in_maps = _host_prep(bitstrings, theta, idx_pad)
    nc = _get_module()
    res = run_bass_kernel_spmd(nc, in_maps, core_ids=list(range(N_CORES)))
    out = np.concatenate([_unpack_out(r["out"]) for r in res.results])
    return out.astype(np.float32)


# revision 37
# speedup vs baseline: 1.0431x; 1.0431x over previous
"""Trainium2 Bass kernel: parity-polynomial segment_reduce (one-hot form).

Reference math:
    spins = 1 - 2*bits                                   # {-1,+1}
    parities[b,t] = prod_o spins_pad[b, idx_pad[t,o]]    # [B, T]
    out[b] = parities[b] @ theta

Every parity factor is (-1)^{bit}, so with mask[t] = XOR-fold of idx_pad[t]
bit positions (pad index contributes nothing, repeats cancel):

    out[b] = f(key_b),   f = WHT_4096(theta scattered by mask)

where key_b is the 12-bit key from bits 0..11 (all masks span bits 0..11).
Asymmetric split key = kh*128 + kl (7 low bits, 5 high bits). Then
out[b] = F[kl_b, kh_b] with
F[k,j] = sum_{ml,mh} Theta[mh,ml] * H128[ml,k] * H32[mh,j].

On device (per core, batch-sharded 512 rows, 4 chunks of 128 samples):

  1. key offsets by matmul: M1[k,b] = kl(b)-k  (augmented ones row carries
     the -k), and transposed M2T[b,j] = kh(b)-j per 128-sample chunk
     (stationary = bits chunk).  All operands share base partition 0 by
     packing lo/hi augmented bits and patterns in separate column ranges
     of one 8-partition tile.
  2. one-hots via DVE is_equal against immediate 0.0 (proven ISA form).
  3. F chain (theta-gated, parallel to 1-2): B = H32-contraction of
     Theta, F = H128-contraction of B -- two matmuls with ACT-engine
     PSUM->SBUF bf16 copies between (GPSIMD cannot access PSUM).
  4. RT[b,j] = F[kl(b), j]: 4 matmuls with the one-hot chunk as the
     stationary operand, so RT lands batch-major [128, 4*32].
  5. out = sum_j RT * ohhiT: DVE tensor_mul + free-dim tensor_reduce
     -> y[128,4] partition-major, sample g*128+j on partition j, col g.
  6. store via SWDGE kv_writeback whose descriptors are PREPARED during
     the input-DMA wait; the trigger (manually gated on a semaphore the
     reduce's follower bumps) skips the HWDGE and DGE-delay latencies of
     a fresh DMA.  An SP wait on the DMA-completion semaphore gates
     kernel exit.

PE warm-up matmuls (junk outer products) run during the input DMA wait so
the real matmuls hit a warm PE pipeline on hardware.

Host does only sharding, dtype/layout staging, and index bookkeeping
(mask XOR-fold + theta scatter).  All theta- and bit-dependent arithmetic
runs on device.
"""

import numpy as np

B, NUM_BITS, ORDER = 4096, 32, 12
N_CORES = 8
B_LOCAL = B // N_CORES          # 512
KEYS = 1 << ORDER               # 4096
LO, HI = 7, 5                   # asymmetric key split
KL, KH = 1 << LO, 1 << HI       # 128, 32
ROWS = LO + 1                   # lo bit rows + ones row (tile height)
CHUNK = 128                     # samples per transposed chunk
N_CHUNK = B_LOCAL // CHUNK      # 4
TB_COLS = 2 * B_LOCAL + KL + KH  # bits-lo | bits-hi | patL | patH
N_WARM = 18                     # PE warm-up matmuls (128 cols each)

_STATE = {}


def _sylvester(n):
    """H[i,j] = (-1)^popcount(i&j), Sylvester ordering."""
    h = np.array([[1.0]], dtype=np.float32)
    while h.shape[0] < n:
        h = np.block([[h, h], [h, -h]])
    return np.ascontiguousarray(h, dtype=np.float32)


def _build_module():
    import concourse.mybir as mybir
    import concourse.tile as tile
    from concourse import bacc

    f32 = mybir.dt.float32
    bf16 = mybir.dt.bfloat16
    i32 = mybir.dt.int32
    nc = bacc.Bacc(
        "TRN2",
        target_bir_lowering=False,
        debug=False,
        enable_asserts=True,
        num_devices=N_CORES,
    )

    # tb [8, 1184]: cols 0..511 lo-augmented bits (rows 0..6 bits 0..6,
    # row 7 ones), cols 512..1023 hi-augmented bits (rows 0..4 bits 7..11,
    # row 5 ones), cols 1024..1151 patL (rows 0..6 = 1<<i, row 7 = -k),
    # cols 1152..1183 patH (rows 0..4 = 1<<i, row 5 = -j).
    tb = nc.dram_tensor("tb", [ROWS, TB_COLS], bf16, kind="ExternalInput").ap()
    # tt [128, 288]: cols 0..127 H128, cols 128..255 Theta[mh,ml] (rows
    # 0..31), cols 256..287 H32 (rows 0..31).
    tt = nc.dram_tensor("tt", [KL, KL + KL + KH], bf16, kind="ExternalInput").ap()
    # kv_writeback layout [batch=1, d_head_inner=128, d_head_outer=1, n_ctx=4]
    out = nc.dram_tensor("out", [1, CHUNK, 1, N_CHUNK], f32, kind="ExternalOutput").ap()

    with tile.TileContext(nc) as tc:
        with (
            tc.tile_pool(name="sb", bufs=1) as sb,
            tc.tile_pool(name="ps", bufs=1, space="PSUM") as ps,
        ):
            # --- warm-up fuel + output index metadata (no input deps) ---
            t_wu = sb.tile([1, CHUNK], bf16)
            nc.vector.memset(t_wu, 1.0)
            t_ctx = sb.tile([CHUNK, 1], i32)
            nc.gpsimd.memset(t_ctx, 0)

            # --- input DMAs: bits first, theta second ---
            t_b = sb.tile([ROWS, TB_COLS], bf16)
            nc.sync.dma_start(out=t_b, in_=tb)
            t_t = sb.tile([KL, KL + KL + KH], bf16)
            nc.sync.dma_start(out=t_t, in_=tt)

            # --- result tile + prepared output descriptors ---
            # Emitted before t_y has any writer, so the prep's only sync dep
            # is the t_ctx memset and it runs during the input-DMA wait.
            t_y = sb.tile([CHUNK, N_CHUNK], f32)
            dma_sem = nc.alloc_semaphore("swdge_dma")
            sem_y = nc.alloc_semaphore("y_ready")
            y2d = t_y[:, :]
            from concourse.ap import AP as _AP

            # [dhi=128, dho=1, batch=1, ncn=4]; dho stride 4 so batch_step=1
            in4d = _AP(
                y2d.tensor,
                y2d.offset,
                [list(y2d.ap[0]), [N_CHUNK, 1], [N_CHUNK, 1], [1, N_CHUNK]],
            )
            nc.gpsimd.kv_writeback(
                out, in4d, t_ctx[:, :], prepare_only=True, sem=dma_sem
            )

            # --- PE warm-up: junk outer products while the DMA flies ---
            p_wu = ps.tile([CHUNK, CHUNK], f32)
            for _ in range(N_WARM):
                nc.tensor.matmul(p_wu, t_wu, t_wu)

            bits_lo = t_b[:, 0:B_LOCAL]
            bits_hi = t_b[0:HI + 1, B_LOCAL : 2 * B_LOCAL]
            patL = t_b[:, 2 * B_LOCAL : 2 * B_LOCAL + KL]
            patH = t_b[0:HI + 1, 2 * B_LOCAL + KL : 2 * B_LOCAL + KL + KH]
            t_H128 = t_t[:, 0:KL]
            t_Th = t_t[0:KH, KL : 2 * KL]
            t_H32 = t_t[0:KH, 2 * KL : 2 * KL + KH]

            # --- key-offset matmuls ---
            p_M1 = ps.tile([KL, B_LOCAL], f32)
            nc.tensor.matmul(p_M1, patL, bits_lo)          # kl(b) - k
            p_M2T = ps.tile([CHUNK, N_CHUNK * KH], f32)
            for g in range(N_CHUNK):
                nc.tensor.matmul(
                    p_M2T[:, g * KH : (g + 1) * KH],
                    bits_hi[:, g * CHUNK : (g + 1) * CHUNK],
                    patH,
                )                                          # kh(b) - j

            # --- F chain (theta-gated; ACT copies keep DVE free) ---
            p_B = ps.tile([KL, KH], f32)
            nc.tensor.matmul(p_B, t_Th, t_H32)             # B[ml,j]
            t_B = sb.tile([KL, KH], bf16)
            nc.scalar.copy(t_B, p_B)
            p_F = ps.tile([KL, KH], f32)
            nc.tensor.matmul(p_F, t_H128, t_B)             # F[k,j]
            t_F = sb.tile([KL, KH], bf16)
            nc.scalar.copy(t_F, p_F)

            # --- one-hots (DVE is_equal vs immediate 0.0) ---
            def onehot(dst, src):
                nc.vector.tensor_scalar(
                    out=dst,
                    in0=src,
                    scalar1=0.0,
                    scalar2=None,
                    op0=mybir.AluOpType.is_equal,
                )

            t_ohlo = sb.tile([KL, B_LOCAL], bf16)
            onehot(t_ohlo, p_M1)
            t_ohhiT = sb.tile([CHUNK, N_CHUNK * KH], bf16)
            onehot(t_ohhiT, p_M2T)

            # --- RT[b,j] = F[kl(b), j] via stationary one-hot chunks ---
            p_RT = ps.tile([CHUNK, N_CHUNK * KH], f32)
            for g in range(N_CHUNK):
                nc.tensor.matmul(
                    p_RT[:, g * KH : (g + 1) * KH],
                    t_ohlo[:, g * CHUNK : (g + 1) * CHUNK],
                    t_F,
                )

            # --- combine + reduce -> y[j, g] = out[g*128 + j] ---
            t_prod = sb.tile([CHUNK, N_CHUNK * KH], bf16)
            nc.vector.tensor_mul(t_prod, p_RT, t_ohhiT)
            red = nc.vector.tensor_reduce(
                out=t_y[:, :],
                in_=t_prod[:, :].rearrange("p (g m) -> p g m", m=KH),
                axis=mybir.AxisListType.X,
                op=mybir.AluOpType.add,
            )
            # separate DVE sem bump (engine ops can't carry 2 updates); the
            # sync edge pins it after the reduce so the scheduler can't hoist
            import bass_rust as _br

            si = nc.vector.sem_inc(sem_y, 1)
            _deps = _br.InstructionNameOrderedSet()
            _deps.add(red.ins.name)
            si.ins.add_sync_dependencies_from(_deps)

            # --- fire the prepared writeback once t_y is real ---
            nc.gpsimd.trigger_dma(count=1)._wait_ge(sem_y, 1)
            # gate kernel exit on the writeback landing in DRAM (idle SP)
            nc.sync.wait_ge(dma_sem, 16)

    nc.compile()
    return nc


def _get_module():
    nc = _STATE.get("nc")
    if nc is None:
        nc = _build_module()
        _STATE["nc"] = nc
    return nc


def _host_prep(bitstrings, theta, idx_pad):
    """Index bookkeeping + input staging. Returns per-core input maps."""
    import ml_dtypes

    bitstrings = np.asarray(bitstrings)
    theta = np.asarray(theta, dtype=np.float32)
    idx_pad = np.asarray(idx_pad).astype(np.int64)

    # mask[t] = XOR-fold of one-hot bit positions (pad index >= NUM_BITS -> no bit)
    onehots = np.where(idx_pad >= NUM_BITS, 0, np.int64(1) << np.clip(idx_pad, 0, 62))
    masks = np.bitwise_xor.reduce(onehots, axis=1)
    if masks.size and int(masks.max()) >= KEYS:
        raise NotImplementedError(
            "kernel specialized for masks spanning bits 0..11 "
            f"(max mask {int(masks.max())})"
        )
    theta_spread = np.zeros(KEYS, np.float32)
    np.add.at(theta_spread, masks, theta)

    # H128 | Theta[mh, ml] | H32
    ttbuf = np.zeros((KL, KL + KL + KH), np.float32)
    ttbuf[:, 0:KL] = _sylvester(KL)
    ttbuf[0:KH, KL : 2 * KL] = theta_spread.reshape(KH, KL)
    ttbuf[0:KH, 2 * KL : 2 * KL + KH] = _sylvester(KH)
    tt = ttbuf.astype(ml_dtypes.bfloat16)

    # patterns: patL[i,k] = 1<<i, patL[7,k] = -k; patH[i,j] = 1<<i,
    # patH[5,j] = -j
    patL = np.zeros((ROWS, KL), np.float32)
    for i in range(LO):
        patL[i, :] = float(1 << i)
    patL[LO, :] = -np.arange(KL, dtype=np.float32)
    patH = np.zeros((HI + 1, KH), np.float32)
    for i in range(HI):
        patH[i, :] = float(1 << i)
    patH[HI, :] = -np.arange(KH, dtype=np.float32)

    bits_f = bitstrings.astype(np.float32)
    in_maps = []
    for c in range(N_CORES):
        bl = bits_f[c * B_LOCAL : (c + 1) * B_LOCAL, :]  # [512, 32]
        tbuf = np.zeros((ROWS, TB_COLS), np.float32)
        tbuf[0:LO, 0:B_LOCAL] = bl[:, 0:LO].T                     # bits 0..6
        tbuf[LO, 0:B_LOCAL] = 1.0                                 # ones row
        tbuf[0:HI, B_LOCAL : 2 * B_LOCAL] = bl[:, LO:ORDER].T     # bits 7..11
        tbuf[HI, B_LOCAL : 2 * B_LOCAL] = 1.0                     # ones row
        tbuf[:, 2 * B_LOCAL : 2 * B_LOCAL + KL] = patL
        tbuf[0 : HI + 1, 2 * B_LOCAL + KL :] = patH
        in_maps.append({"tb": tbuf.astype(ml_dtypes.bfloat16), "tt": tt})
    return in_maps


def _unpack_out(arr):
    """[1,128,1,4] device layout -> [512] local outputs (b = g*128 + j)."""
    a = np.asarray(arr, dtype=np.float32).reshape(CHUNK, N_CHUNK)
    return a.T.reshape(-1)


def kernel(bitstrings, theta, idx_pad):
    from concourse.bass_utils import run_bass_kernel_spmd

    in_maps = _host_prep(bitstrings, theta, idx_pad)
    nc = _get_module()
    res = run_bass_kernel_spmd(nc, in_maps, core_ids=list(range(N_CORES)))
    out = np.concatenate([_unpack_out(r["out"]) for r in res.results])
    return out.astype(np.float32)


# revision 38
# speedup vs baseline: 1.0512x; 1.0077x over previous
"""Trainium2 Bass kernel: parity-polynomial segment_reduce (one-hot form).

Reference math:
    spins = 1 - 2*bits                                   # {-1,+1}
    parities[b,t] = prod_o spins_pad[b, idx_pad[t,o]]    # [B, T]
    out[b] = parities[b] @ theta

Every parity factor is (-1)^{bit}, so with mask[t] = XOR-fold of idx_pad[t]
bit positions (pad index contributes nothing, repeats cancel):

    out[b] = f(key_b),   f = WHT_4096(theta scattered by mask)

where key_b is the 12-bit key from bits 0..11 (all masks span bits 0..11).
Asymmetric split key = kh*128 + kl (7 low bits, 5 high bits). Then
out[b] = F[kl_b, kh_b] with
F[k,j] = sum_{ml,mh} Theta[mh,ml] * H128[ml,k] * H32[mh,j].

On device (per core, batch-sharded 512 rows, 4 chunks of 128 samples):

  1. key offsets by matmul: M1[k,b] = kl(b)-k  (augmented ones row carries
     the -k), and transposed M2T[b,j] = kh(b)-j per 128-sample chunk
     (stationary = bits chunk).  All operands share base partition 0 by
     packing lo/hi augmented bits and patterns in separate column ranges
     of one 8-partition tile.
  2. one-hots via DVE is_equal against immediate 0.0 (proven ISA form).
  3. F chain (theta-gated, parallel to 1-2): B = H32-contraction of
     Theta, F = H128-contraction of B -- two matmuls with ACT-engine
     PSUM->SBUF bf16 copies between (GPSIMD cannot access PSUM).
  4. RT[b,j] = F[kl(b), j]: 4 matmuls with the one-hot chunk as the
     stationary operand, so RT lands batch-major [128, 4*32].
  5. out = sum_j RT * ohhiT: DVE tensor_mul + free-dim tensor_reduce
     -> y[128,4] partition-major, sample g*128+j on partition j, col g.
  6. store via SWDGE kv_writeback whose descriptors are PREPARED during
     the input-DMA wait; the trigger (manually gated on a semaphore the
     reduce's follower bumps) skips the HWDGE and DGE-delay latencies of
     a fresh DMA.  An SP wait on the DMA-completion semaphore gates
     kernel exit.

PE warm-up matmuls (junk outer products) run during the input DMA wait so
the real matmuls hit a warm PE pipeline on hardware.

Host does only sharding, dtype/layout staging, and index bookkeeping
(mask XOR-fold + theta scatter).  All theta- and bit-dependent arithmetic
runs on device.
"""

import numpy as np

B, NUM_BITS, ORDER = 4096, 32, 12
N_CORES = 8
B_LOCAL = B // N_CORES          # 512
KEYS = 1 << ORDER               # 4096
LO, HI = 7, 5                   # asymmetric key split
KL, KH = 1 << LO, 1 << HI       # 128, 32
ROWS = LO + 1                   # lo bit rows + ones row (tile height)
CHUNK = 128                     # samples per transposed chunk
N_CHUNK = B_LOCAL // CHUNK      # 4
TB_COLS = 2 * B_LOCAL + KL + KH  # bits-lo | bits-hi | patL | patH
N_WARM = 18                     # PE warm-up matmuls (128 cols each)

_STATE = {}


def _sylvester(n):
    """H[i,j] = (-1)^popcount(i&j), Sylvester ordering."""
    h = np.array([[1.0]], dtype=np.float32)
    while h.shape[0] < n:
        h = np.block([[h, h], [h, -h]])
    return np.ascontiguousarray(h, dtype=np.float32)


def _build_module():
    import concourse.mybir as mybir
    import concourse.tile as tile
    from concourse import bacc

    f32 = mybir.dt.float32
    bf16 = mybir.dt.bfloat16
    i32 = mybir.dt.int32
    nc = bacc.Bacc(
        "TRN2",
        target_bir_lowering=False,
        debug=False,
        enable_asserts=True,
        num_devices=N_CORES,
    )

    # tb [8, 1184]: cols 0..511 lo-augmented bits (rows 0..6 bits 0..6,
    # row 7 ones), cols 512..1023 hi-augmented bits (rows 0..4 bits 7..11,
    # row 5 ones), cols 1024..1151 patL (rows 0..6 = 1<<i, row 7 = -k),
    # cols 1152..1183 patH (rows 0..4 = 1<<i, row 5 = -j).
    tb = nc.dram_tensor("tb", [ROWS, TB_COLS], bf16, kind="ExternalInput").ap()
    # tt [128, 288]: cols 0..127 H128, cols 128..255 Theta[mh,ml] (rows
    # 0..31), cols 256..287 H32 (rows 0..31).
    tt = nc.dram_tensor("tt", [KL, KL + KL + KH], bf16, kind="ExternalInput").ap()
    # kv_writeback layout [batch=1, d_head_inner=128, d_head_outer=1, n_ctx=4]
    out = nc.dram_tensor("out", [1, CHUNK, 1, N_CHUNK], f32, kind="ExternalOutput").ap()

    with tile.TileContext(nc) as tc:
        with (
            tc.tile_pool(name="sb", bufs=1) as sb,
            tc.tile_pool(name="ps", bufs=1, space="PSUM") as ps,
        ):
            # --- warm-up fuel + output index metadata (no input deps) ---
            t_wu = sb.tile([1, CHUNK], bf16)
            nc.vector.memset(t_wu, 1.0)
            t_ctx = sb.tile([CHUNK, 1], i32)
            nc.gpsimd.memset(t_ctx, 0)

            # --- input DMAs: bits on SP/HWDGE, theta on Pool/SWDGE so the
            # two don't serialize on the SP sequencer + HWDGE ---
            t_b = sb.tile([ROWS, TB_COLS], bf16)
            nc.sync.dma_start(out=t_b, in_=tb)
            t_t = sb.tile([KL, KL + KL + KH], bf16)
            nc.gpsimd.dma_start(out=t_t, in_=tt)

            # --- result tile + prepared output descriptors ---
            # Emitted before t_y has any writer, so the prep's only sync dep
            # is the t_ctx memset and it runs during the input-DMA wait.
            t_y = sb.tile([CHUNK, N_CHUNK], f32)
            dma_sem = nc.alloc_semaphore("swdge_dma")
            sem_y = nc.alloc_semaphore("y_ready")
            y2d = t_y[:, :]
            from concourse.ap import AP as _AP

            # [dhi=128, dho=1, batch=1, ncn=4]; dho stride 4 so batch_step=1
            in4d = _AP(
                y2d.tensor,
                y2d.offset,
                [list(y2d.ap[0]), [N_CHUNK, 1], [N_CHUNK, 1], [1, N_CHUNK]],
            )
            nc.gpsimd.kv_writeback(
                out, in4d, t_ctx[:, :], prepare_only=True, sem=dma_sem
            )

            # --- PE warm-up: junk outer products while the DMA flies ---
            p_wu = ps.tile([CHUNK, CHUNK], f32)
            for _ in range(N_WARM):
                nc.tensor.matmul(p_wu, t_wu, t_wu)

            bits_lo = t_b[:, 0:B_LOCAL]
            bits_hi = t_b[0:HI + 1, B_LOCAL : 2 * B_LOCAL]
            patL = t_b[:, 2 * B_LOCAL : 2 * B_LOCAL + KL]
            patH = t_b[0:HI + 1, 2 * B_LOCAL + KL : 2 * B_LOCAL + KL + KH]
            t_H128 = t_t[:, 0:KL]
            t_Th = t_t[0:KH, KL : 2 * KL]
            t_H32 = t_t[0:KH, 2 * KL : 2 * KL + KH]

            # --- key-offset matmuls ---
            p_M1 = ps.tile([KL, B_LOCAL], f32)
            nc.tensor.matmul(p_M1, patL, bits_lo)          # kl(b) - k
            p_M2T = ps.tile([CHUNK, N_CHUNK * KH], f32)
            for g in range(N_CHUNK):
                nc.tensor.matmul(
                    p_M2T[:, g * KH : (g + 1) * KH],
                    bits_hi[:, g * CHUNK : (g + 1) * CHUNK],
                    patH,
                )                                          # kh(b) - j

            # --- F chain (theta-gated; ACT copies keep DVE free) ---
            p_B = ps.tile([KL, KH], f32)
            nc.tensor.matmul(p_B, t_Th, t_H32)             # B[ml,j]
            t_B = sb.tile([KL, KH], bf16)
            nc.scalar.copy(t_B, p_B)
            p_F = ps.tile([KL, KH], f32)
            nc.tensor.matmul(p_F, t_H128, t_B)             # F[k,j]
            t_F = sb.tile([KL, KH], bf16)
            nc.scalar.copy(t_F, p_F)

            # --- one-hots (DVE is_equal vs immediate 0.0) ---
            def onehot(dst, src):
                nc.vector.tensor_scalar(
                    out=dst,
                    in0=src,
                    scalar1=0.0,
                    scalar2=None,
                    op0=mybir.AluOpType.is_equal,
                )

            t_ohlo = sb.tile([KL, B_LOCAL], bf16)
            onehot(t_ohlo, p_M1)
            t_ohhiT = sb.tile([CHUNK, N_CHUNK * KH], bf16)
            onehot(t_ohhiT, p_M2T)

            # --- RT[b,j] = F[kl(b), j] via stationary one-hot chunks ---
            p_RT = ps.tile([CHUNK, N_CHUNK * KH], f32)
            for g in range(N_CHUNK):
                nc.tensor.matmul(
                    p_RT[:, g * KH : (g + 1) * KH],
                    t_ohlo[:, g * CHUNK : (g + 1) * CHUNK],
                    t_F,
                )

            # --- combine + reduce -> y[j, g] = out[g*128 + j] ---
            t_prod = sb.tile([CHUNK, N_CHUNK * KH], bf16)
            nc.vector.tensor_mul(t_prod, p_RT, t_ohhiT)
            red = nc.vector.tensor_reduce(
                out=t_y[:, :],
                in_=t_prod[:, :].rearrange("p (g m) -> p g m", m=KH),
                axis=mybir.AxisListType.X,
                op=mybir.AluOpType.add,
            )
            # separate DVE sem bump (engine ops can't carry 2 updates); the
            # sync edge pins it after the reduce so the scheduler can't hoist
            import bass_rust as _br

            si = nc.vector.sem_inc(sem_y, 1)
            _deps = _br.InstructionNameOrderedSet()
            _deps.add(red.ins.name)
            si.ins.add_sync_dependencies_from(_deps)

            # --- fire the prepared writeback once t_y is real ---
            nc.gpsimd.trigger_dma(count=1)._wait_ge(sem_y, 1)
            # gate kernel exit on the writeback landing in DRAM (idle SP)
            nc.sync.wait_ge(dma_sem, 16)

    nc.compile()
    return nc


def _get_module():
    nc = _STATE.get("nc")
    if nc is None:
        nc = _build_module()
        _STATE["nc"] = nc
    return nc


def _host_prep(bitstrings, theta, idx_pad):
    """Index bookkeeping + input staging. Returns per-core input maps."""
    import ml_dtypes

    bitstrings = np.asarray(bitstrings)
    theta = np.asarray(theta, dtype=np.float32)
    idx_pad = np.asarray(idx_pad).astype(np.int64)

    # mask[t] = XOR-fold of one-hot bit positions (pad index >= NUM_BITS -> no bit)
    onehots = np.where(idx_pad >= NUM_BITS, 0, np.int64(1) << np.clip(idx_pad, 0, 62))
    masks = np.bitwise_xor.reduce(onehots, axis=1)
    if masks.size and int(masks.max()) >= KEYS:
        raise NotImplementedError(
            "kernel specialized for masks spanning bits 0..11 "
            f"(max mask {int(masks.max())})"
        )
    theta_spread = np.zeros(KEYS, np.float32)
    np.add.at(theta_spread, masks, theta)

    # H128 | Theta[mh, ml] | H32
    ttbuf = np.zeros((KL, KL + KL + KH), np.float32)
    ttbuf[:, 0:KL] = _sylvester(KL)
    ttbuf[0:KH, KL : 2 * KL] = theta_spread.reshape(KH, KL)
    ttbuf[0:KH, 2 * KL : 2 * KL + KH] = _sylvester(KH)
    tt = ttbuf.astype(ml_dtypes.bfloat16)

    # patterns: patL[i,k] = 1<<i, patL[7,k] = -k; patH[i,j] = 1<<i,
    # patH[5,j] = -j
    patL = np.zeros((ROWS, KL), np.float32)
    for i in range(LO):
        patL[i, :] = float(1 << i)
    patL[LO, :] = -np.arange(KL, dtype=np.float32)
    patH = np.zeros((HI + 1, KH), np.float32)
    for i in range(HI):
        patH[i, :] = float(1 << i)
    patH[HI, :] = -np.arange(KH, dtype=np.float32)

    bits_f = bitstrings.astype(np.float32)
    in_maps = []
    for c in range(N_CORES):
        bl = bits_f[c * B_LOCAL : (c + 1) * B_LOCAL, :]  # [512, 32]
        tbuf = np.zeros((ROWS, TB_COLS), np.float32)
        tbuf[0:LO, 0:B_LOCAL] = bl[:, 0:LO].T                     # bits 0..6
        tbuf[LO, 0:B_LOCAL] = 1.0                                 # ones row
        tbuf[0:HI, B_LOCAL : 2 * B_LOCAL] = bl[:, LO:ORDER].T     # bits 7..11
        tbuf[HI, B_LOCAL : 2 * B_LOCAL] = 1.0                     # ones row
        tbuf[:, 2 * B_LOCAL : 2 * B_LOCAL + KL] = patL
        tbuf[0 : HI + 1, 2 * B_LOCAL + KL :] = patH
        in_maps.append({"tb": tbuf.astype(ml_dtypes.bfloat16), "tt": tt})
    return in_maps


def _unpack_out(arr):
    """[1,128,1,4] device layout -> [512] local outputs (b = g*128 + j)."""
    a = np.asarray(arr, dtype=np.float32).reshape(CHUNK, N_CHUNK)
    return a.T.reshape(-1)


def kernel(bitstrings, theta, idx_pad):
    from concourse.bass_utils import run_bass_kernel_spmd

    in_maps = _host_prep(bitstrings, theta, idx_pad)
    nc = _get_module()
    res = run_bass_kernel_spmd(nc, in_maps, core_ids=list(range(N_CORES)))
    out = np.concatenate([_unpack_out(r["out"]) for r in res.results])
    return out.astype(np.float32)


# revision 39
# speedup vs baseline: 1.0644x; 1.0126x over previous
"""Trainium2 Bass kernel: parity-polynomial segment_reduce (one-hot form).

Reference math:
    spins = 1 - 2*bits                                   # {-1,+1}
    parities[b,t] = prod_o spins_pad[b, idx_pad[t,o]]    # [B, T]
    out[b] = parities[b] @ theta

Every parity factor is (-1)^{bit}, so with mask[t] = XOR-fold of idx_pad[t]
bit positions (pad index contributes nothing, repeats cancel):

    out[b] = f(key_b),   f = WHT_4096(theta scattered by mask)

where key_b is the 12-bit key from bits 0..11 (all masks span bits 0..11).
Asymmetric split key = kh*128 + kl (7 low bits, 5 high bits). Then
out[b] = F[kl_b, kh_b] with
F[k,j] = sum_{ml,mh} Theta[mh,ml] * H128[ml,k] * H32[mh,j].

On device (per core, batch-sharded 512 rows, 4 chunks of 128 samples):

  1. key offsets by matmul: M1[k,b] = kl(b)-k  (augmented ones row carries
     the -k), and transposed M2T[b,j] = kh(b)-j per 128-sample chunk
     (stationary = bits chunk).  All operands share base partition 0 by
     packing lo/hi augmented bits and patterns in separate column ranges
     of one 8-partition tile.
  2. one-hots via DVE is_equal against immediate 0.0 (proven ISA form).
  3. F chain (theta-gated, parallel to 1-2): B = H32-contraction of
     Theta, F = H128-contraction of B -- two matmuls with ACT-engine
     PSUM->SBUF bf16 copies between (GPSIMD cannot access PSUM).
  4. RT[b,j] = F[kl(b), j]: 4 matmuls with the one-hot chunk as the
     stationary operand, so RT lands batch-major [128, 4*32].
  5. out = sum_j RT * ohhiT: DVE tensor_mul + free-dim tensor_reduce
     -> y[128,4] partition-major, sample g*128+j on partition j, col g.
  6. store via SWDGE kv_writeback whose descriptors are PREPARED during
     the input-DMA wait; the trigger (manually gated on a semaphore the
     reduce's follower bumps) skips the HWDGE and DGE-delay latencies of
     a fresh DMA.  An SP wait on the DMA-completion semaphore gates
     kernel exit.

PE warm-up matmuls (junk outer products) run during the input DMA wait so
the real matmuls hit a warm PE pipeline on hardware.

Host does only sharding, dtype/layout staging, and index bookkeeping
(mask XOR-fold + theta scatter).  All theta- and bit-dependent arithmetic
runs on device.
"""

import numpy as np

B, NUM_BITS, ORDER = 4096, 32, 12
N_CORES = 8
B_LOCAL = B // N_CORES          # 512
KEYS = 1 << ORDER               # 4096
LO, HI = 7, 5                   # asymmetric key split
KL, KH = 1 << LO, 1 << HI       # 128, 32
ROWS = LO + 1                   # lo bit rows + ones row (tile height)
CHUNK = 128                     # samples per transposed chunk
N_CHUNK = B_LOCAL // CHUNK      # 4
TB_COLS = 2 * B_LOCAL + KL + KH  # bits-lo | bits-hi | patL | patH
N_WARM = 18                     # PE warm-up matmuls (128 cols each)

_STATE = {}


def _sylvester(n):
    """H[i,j] = (-1)^popcount(i&j), Sylvester ordering."""
    h = np.array([[1.0]], dtype=np.float32)
    while h.shape[0] < n:
        h = np.block([[h, h], [h, -h]])
    return np.ascontiguousarray(h, dtype=np.float32)


def _build_module():
    import concourse.mybir as mybir
    import concourse.tile as tile
    from concourse import bacc

    f32 = mybir.dt.float32
    bf16 = mybir.dt.bfloat16
    i32 = mybir.dt.int32
    nc = bacc.Bacc(
        "TRN2",
        target_bir_lowering=False,
        debug=False,
        enable_asserts=True,
        num_devices=N_CORES,
    )

    # tb [8, 1184]: cols 0..511 lo-augmented bits (rows 0..6 bits 0..6,
    # row 7 ones), cols 512..1023 hi-augmented bits (rows 0..4 bits 7..11,
    # row 5 ones), cols 1024..1151 patL (rows 0..6 = 1<<i, row 7 = -k),
    # cols 1152..1183 patH (rows 0..4 = 1<<i, row 5 = -j).
    tb = nc.dram_tensor("tb", [ROWS, TB_COLS], bf16, kind="ExternalInput").ap()
    # tt [128, 288]: cols 0..127 H128, cols 128..255 Theta[mh,ml] (rows
    # 0..31), cols 256..287 H32 (rows 0..31).
    tt = nc.dram_tensor("tt", [KL, KL + KL + KH], bf16, kind="ExternalInput").ap()
    # kv_writeback layout [batch=1, d_head_inner=128, d_head_outer=1, n_ctx=4]
    out = nc.dram_tensor("out", [1, CHUNK, 1, N_CHUNK], f32, kind="ExternalOutput").ap()

    with tile.TileContext(nc) as tc:
        with (
            tc.tile_pool(name="sb", bufs=1) as sb,
            tc.tile_pool(name="ps", bufs=1, space="PSUM") as ps,
        ):
            # --- input DMAs: bits on SP/HWDGE, theta on Pool/SWDGE so the
            # two don't serialize on the SP sequencer + HWDGE.  The theta
            # DMA is Pool's first op so its descriptor gen starts at once.
            t_b = sb.tile([ROWS, TB_COLS], bf16)
            nc.sync.dma_start(out=t_b, in_=tb)
            t_t = sb.tile([KL, KL + KL + KH], bf16)
            nc.gpsimd.dma_start(out=t_t, in_=tt)

            # --- warm-up fuel + output index metadata (no input deps) ---
            t_wu = sb.tile([1, CHUNK], bf16)
            nc.vector.memset(t_wu, 1.0)
            t_ctx = sb.tile([CHUNK, 1], i32)
            nc.gpsimd.memset(t_ctx, 0)

            # --- result tile + prepared output descriptors ---
            # Emitted before t_y has any writer, so the prep's only sync dep
            # is the t_ctx memset and it runs during the input-DMA wait.
            t_y = sb.tile([CHUNK, N_CHUNK], f32)
            dma_sem = nc.alloc_semaphore("swdge_dma")
            sem_y = nc.alloc_semaphore("y_ready")
            y2d = t_y[:, :]
            from concourse.ap import AP as _AP

            # [dhi=128, dho=1, batch=1, ncn=4]; dho stride 4 so batch_step=1
            in4d = _AP(
                y2d.tensor,
                y2d.offset,
                [list(y2d.ap[0]), [N_CHUNK, 1], [N_CHUNK, 1], [1, N_CHUNK]],
            )
            nc.gpsimd.kv_writeback(
                out, in4d, t_ctx[:, :], prepare_only=True, sem=dma_sem
            )

            # --- PE warm-up: junk outer products while the DMA flies ---
            p_wu = ps.tile([CHUNK, CHUNK], f32)
            for _ in range(N_WARM):
                nc.tensor.matmul(p_wu, t_wu, t_wu)

            bits_lo = t_b[:, 0:B_LOCAL]
            bits_hi = t_b[0:HI + 1, B_LOCAL : 2 * B_LOCAL]
            patL = t_b[:, 2 * B_LOCAL : 2 * B_LOCAL + KL]
            patH = t_b[0:HI + 1, 2 * B_LOCAL + KL : 2 * B_LOCAL + KL + KH]
            t_H128 = t_t[:, 0:KL]
            t_Th = t_t[0:KH, KL : 2 * KL]
            t_H32 = t_t[0:KH, 2 * KL : 2 * KL + KH]

            # --- key-offset matmuls ---
            p_M1 = ps.tile([KL, B_LOCAL], f32)
            nc.tensor.matmul(p_M1, patL, bits_lo)          # kl(b) - k
            p_M2T = ps.tile([CHUNK, N_CHUNK * KH], f32)
            for g in range(N_CHUNK):
                nc.tensor.matmul(
                    p_M2T[:, g * KH : (g + 1) * KH],
                    bits_hi[:, g * CHUNK : (g + 1) * CHUNK],
                    patH,
                )                                          # kh(b) - j

            # --- F chain (theta-gated; ACT copies keep DVE free) ---
            p_B = ps.tile([KL, KH], f32)
            nc.tensor.matmul(p_B, t_Th, t_H32)             # B[ml,j]
            t_B = sb.tile([KL, KH], bf16)
            nc.scalar.copy(t_B, p_B)
            p_F = ps.tile([KL, KH], f32)
            nc.tensor.matmul(p_F, t_H128, t_B)             # F[k,j]
            t_F = sb.tile([KL, KH], bf16)
            nc.scalar.copy(t_F, p_F)

            # --- one-hots (DVE is_equal vs immediate 0.0) ---
            def onehot(dst, src):
                nc.vector.tensor_scalar(
                    out=dst,
                    in0=src,
                    scalar1=0.0,
                    scalar2=None,
                    op0=mybir.AluOpType.is_equal,
                )

            t_ohlo = sb.tile([KL, B_LOCAL], bf16)
            onehot(t_ohlo, p_M1)
            t_ohhiT = sb.tile([CHUNK, N_CHUNK * KH], bf16)
            onehot(t_ohhiT, p_M2T)

            # --- RT[b,j] = F[kl(b), j] via stationary one-hot chunks ---
            p_RT = ps.tile([CHUNK, N_CHUNK * KH], f32)
            for g in range(N_CHUNK):
                nc.tensor.matmul(
                    p_RT[:, g * KH : (g + 1) * KH],
                    t_ohlo[:, g * CHUNK : (g + 1) * CHUNK],
                    t_F,
                )

            # --- combine + reduce -> y[j, g] = out[g*128 + j] ---
            t_prod = sb.tile([CHUNK, N_CHUNK * KH], bf16)
            nc.vector.tensor_mul(t_prod, p_RT, t_ohhiT)
            red = nc.vector.tensor_reduce(
                out=t_y[:, :],
                in_=t_prod[:, :].rearrange("p (g m) -> p g m", m=KH),
                axis=mybir.AxisListType.X,
                op=mybir.AluOpType.add,
            )
            # separate DVE sem bump (engine ops can't carry 2 updates); the
            # sync edge pins it after the reduce so the scheduler can't hoist
            import bass_rust as _br

            si = nc.vector.sem_inc(sem_y, 1)
            _deps = _br.InstructionNameOrderedSet()
            _deps.add(red.ins.name)
            si.ins.add_sync_dependencies_from(_deps)

            # --- fire the prepared writeback once t_y is real ---
            nc.gpsimd.trigger_dma(count=1)._wait_ge(sem_y, 1)
            # gate kernel exit on the writeback landing in DRAM (idle SP)
            nc.sync.wait_ge(dma_sem, 16)

    nc.compile()
    return nc


def _get_module():
    nc = _STATE.get("nc")
    if nc is None:
        nc = _build_module()
        _STATE["nc"] = nc
    return nc


def _host_prep(bitstrings, theta, idx_pad):
    """Index bookkeeping + input staging. Returns per-core input maps."""
    import ml_dtypes

    bitstrings = np.asarray(bitstrings)
    theta = np.asarray(theta, dtype=np.float32)
    idx_pad = np.asarray(idx_pad).astype(np.int64)

    # mask[t] = XOR-fold of one-hot bit positions (pad index >= NUM_BITS -> no bit)
    onehots = np.where(idx_pad >= NUM_BITS, 0, np.int64(1) << np.clip(idx_pad, 0, 62))
    masks = np.bitwise_xor.reduce(onehots, axis=1)
    if masks.size and int(masks.max()) >= KEYS:
        raise NotImplementedError(
            "kernel specialized for masks spanning bits 0..11 "
            f"(max mask {int(masks.max())})"
        )
    theta_spread = np.zeros(KEYS, np.float32)
    np.add.at(theta_spread, masks, theta)

    # H128 | Theta[mh, ml] | H32
    ttbuf = np.zeros((KL, KL + KL + KH), np.float32)
    ttbuf[:, 0:KL] = _sylvester(KL)
    ttbuf[0:KH, KL : 2 * KL] = theta_spread.reshape(KH, KL)
    ttbuf[0:KH, 2 * KL : 2 * KL + KH] = _sylvester(KH)
    tt = ttbuf.astype(ml_dtypes.bfloat16)

    # patterns: patL[i,k] = 1<<i, patL[7,k] = -k; patH[i,j] = 1<<i,
    # patH[5,j] = -j
    patL = np.zeros((ROWS, KL), np.float32)
    for i in range(LO):
        patL[i, :] = float(1 << i)
    patL[LO, :] = -np.arange(KL, dtype=np.float32)
    patH = np.zeros((HI + 1, KH), np.float32)
    for i in range(HI):
        patH[i, :] = float(1 << i)
    patH[HI, :] = -np.arange(KH, dtype=np.float32)

    bits_f = bitstrings.astype(np.float32)
    in_maps = []
    for c in range(N_CORES):
        bl = bits_f[c * B_LOCAL : (c + 1) * B_LOCAL, :]  # [512, 32]
        tbuf = np.zeros((ROWS, TB_COLS), np.float32)
        tbuf[0:LO, 0:B_LOCAL] = bl[:, 0:LO].T                     # bits 0..6
        tbuf[LO, 0:B_LOCAL] = 1.0                                 # ones row
        tbuf[0:HI, B_LOCAL : 2 * B_LOCAL] = bl[:, LO:ORDER].T     # bits 7..11
        tbuf[HI, B_LOCAL : 2 * B_LOCAL] = 1.0                     # ones row
        tbuf[:, 2 * B_LOCAL : 2 * B_LOCAL + KL] = patL
        tbuf[0 : HI + 1, 2 * B_LOCAL + KL :] = patH
        in_maps.append({"tb": tbuf.astype(ml_dtypes.bfloat16), "tt": tt})
    return in_maps


def _unpack_out(arr):
    """[1,128,1,4] device layout -> [512] local outputs (b = g*128 + j)."""
    a = np.asarray(arr, dtype=np.float32).reshape(CHUNK, N_CHUNK)
    return a.T.reshape(-1)


def kernel(bitstrings, theta, idx_pad):
    from concourse.bass_utils import run_bass_kernel_spmd

    in_maps = _host_prep(bitstrings, theta, idx_pad)
    nc = _get_module()
    res = run_bass_kernel_spmd(nc, in_maps, core_ids=list(range(N_CORES)))
    out = np.concatenate([_unpack_out(r["out"]) for r in res.results])
    return out.astype(np.float32)
